# revision 1
# baseline (speedup 1.0000x reference)
"""Trainium2 Bass kernel for nn_CNNInteractLayer (CNN interaction layer).

Math: for each episode b, s-row i, q-row j:
  out[b,i,j] = maxpool_L(relu(conv_k(concat(s[b,i], q[b,j])))) for k in 2..5
Key factorization: conv(concat(s,q)) = conv_s(s) + conv_q(q) + bias, so we
compute per-row convolutions once (25+13 rows per core instead of 625 pairs)
and form pairwise sums with a 0/1 selection matmul on the PE. The max over
the L=31 window runs on the vector engine straight out of PSUM.

Sharding: 8 cores = 4 episodes x 2 halves of the q-row range.
"""

import os
import sys

import numpy as np

for _p in ("/opt/trn_rl_repo",):
    if os.path.isdir(_p) and _p not in sys.path:
        sys.path.insert(0, _p)

# the bass runner needs the axon jax backend; don't let a cpu-only pin hide it
if "axon" not in os.environ.get("JAX_PLATFORMS", "axon"):
    os.environ.pop("JAX_PLATFORMS", None)

from concourse import bacc, bass, mybir, tile  # noqa: E402
from concourse.bass_utils import run_bass_kernel_spmd  # noqa: E402

# Problem dims (hardcoded per spec)
B, N, K, Q, L, D = 4, 5, 5, 5, 31, 512
NROW = N * K            # 25 s-rows per episode
NQROW = N * Q           # 25 q-rows per episode
JN = 13                 # q-rows per core (padded; odd cores use 12)
ROWSTR = L + 4          # padded row stride (pad 2 each side)
POS_S = NROW * ROWSTR   # 875 real positions; computed out to 876 (even chunks)
POS_Q = JN * ROWSTR     # 455; computed out to 456
SLAB_S = 876            # conv output slab width per channel chunk
SLAB_Q = 456
PS_COLS = 880           # input halo: 2 left + enough right for pos 875 + delta 2
PQ_COLS = 460
NCH = 600               # device channels: [k5 | k4 | k3 | k2] x 150
# delta (tap shift) groups; prefix-size in device channel order
DELTAS = [(-2, 300), (-1, 600), (0, 600), (1, 450), (2, 150)]
# emission order per d-chunk: full-coverage groups first so the first matmul
# of each PSUM accumulation group writes the full partition range
DORDER = [1, 2, 0, 3, 4]
WOFF = [0, 300, 900, 1500, 1950]  # packed col offset of each delta group
WSIDE = 2100
CC0 = [0, 128, 256, 384, 512]     # channel chunk starts
CCW = [128, 128, 128, 128, 88]
XROWS = 39                        # 25 s + 13 q + 1 bias
NPAIR = NROW * JN                 # 325
MCH = [(0, 109), (109, 108), (217, 108)]
PAD_OF_K = {2: 1, 3: 1, 4: 2, 5: 2}
ORD_OF_K = {5: 0, 4: 1, 3: 2, 2: 3}
# fp32r matmul requires an even moving-dim size
POSCH_S = [(0, 488), (488, 388)]
POSCH_Q = [(0, 456)]
SUBW = 496                        # pairwise n-subchunk: 16 channel groups

# chunk-major packed-W layout: per channel chunk, [side s | side q], each a
# concatenation of the valid delta groups' column slices for that chunk
def _chunk_tables():
    chw = []          # per-side width of each chunk block
    coloff = {}       # (cc, side, di) -> column offset in packed W
    off = 0
    for cc in range(5):
        c0 = CC0[cc]
        widths = []
        for di, (_, sz) in enumerate(DELTAS):
            w = min(128, sz - c0) if sz > c0 else 0
            widths.append(w)
        side_w = sum(widths)
        for side in range(2):
            p = off + side * side_w
            for di, w in enumerate(widths):
                if w:
                    coloff[(cc, side, di)] = p
                    p += w
        chw.append(side_w)
        off += 2 * side_w
    return chw, coloff


CHW, WCOL = _chunk_tables()
CHOFF = [sum(2 * w for w in CHW[:i]) for i in range(6)]

_PROG = None


def _sub_plan(cc):
    """(offset, width) n-subchunks within an X chunk + psum bank grouping."""
    ccw = CCW[cc]
    total = ccw * 31
    subs = []
    off = 0
    while off < total:
        w = min(SUBW, total - off)
        subs.append((off, w))
        off += w
    # groups of <=3 subchunks sharing one psum tile; equal width within group
    groups = []
    i = 0
    while i < len(subs):
        g = [i]
        while (
            len(g) < 3
            and i + len(g) < len(subs)
            and subs[i + len(g)][1] == subs[i][1]
        ):
            g.append(i + len(g))
        groups.append(g)
        i += len(g)
    return subs, groups


def _build_program():
    nc = bacc.Bacc("TRN2", target_bir_lowering=False, debug=False, num_devices=8)
    f32 = mybir.dt.float32
    f32r = mybir.dt.float32r

    ps_d = nc.dram_tensor("ps", [D, PS_COLS], f32r, kind="ExternalInput")
    pq_d = nc.dram_tensor("pq", [D, PQ_COLS], f32r, kind="ExternalInput")
    w_d = nc.dram_tensor("w", [D, 2 * WSIDE], f32r, kind="ExternalInput")
    a_d = nc.dram_tensor("a", [XROWS, NPAIR], f32r, kind="ExternalInput")
    bias_d = nc.dram_tensor("bias", [1, 5 * 128 * 31], f32r, kind="ExternalInput")
    x_dram = nc.dram_tensor("xstage", [XROWS - 1, 5 * 128 * 31], f32r)
    out_d = nc.dram_tensor("out", [NPAIR, NCH], f32, kind="ExternalOutput")

    with tile.TileContext(nc) as tc:
        with (
            tc.tile_pool(name="persist", bufs=1) as big,
            tc.tile_pool(name="xpool", bufs=3) as xpool,
            tc.tile_pool(name="redpool", bufs=3) as redpool,
            tc.tile_pool(name="convps", bufs=2, space="PSUM") as convps,
            tc.tile_pool(name="pwps", bufs=2, space="PSUM") as pwps,
        ):
            w_sb = big.tile([128, 4 * 2 * WSIDE], f32r, tag="w")
            ps_sb = big.tile([128, 4 * PS_COLS], f32r, tag="ps")
            pq_sb = big.tile([128, 4 * PQ_COLS], f32r, tag="pq")
            cs_sb = big.tile([128, 5 * SLAB_S], f32r, tag="cs")
            cq_sb = big.tile([128, 5 * SLAB_Q], f32r, tag="cq")
            a_sb = big.tile([XROWS, NPAIR], f32r, tag="a")

            # keep the PE busy during the input-DMA prologue so the HAM
            # clock gate is warm (2.4 GHz) when the first conv matmul lands
            warm_sb = big.tile([128, 512], mybir.dt.bfloat16, tag="warm")
            warm_ps = convps.tile([128, 488], f32, tag="conv")
            nc.vector.memset(warm_sb[:], 0.0)
            for _wi in range(80):
                nc.tensor.matmul(
                    warm_ps[0:128, 0:256],
                    lhsT=warm_sb[:, 0:128],
                    rhs=warm_sb[:, 0:256],
                    start=True,
                    stop=True,
                )

            def wload(cc):
                wd = w_d[:].rearrange("(d p) c -> p d c", p=128)
                ws = w_sb[:].rearrange("p (d c) -> p d c", c=2 * WSIDE)
                nc.sync.dma_start(
                    ws[:, :, CHOFF[cc] : CHOFF[cc + 1]],
                    wd[:, :, CHOFF[cc] : CHOFF[cc + 1]],
                )

            wload(0)
            ps3 = ps_sb[:].rearrange("p (d c) -> p d c", c=PS_COLS)
            pd3 = ps_d[:].rearrange("(d p) c -> p d c", p=128)
            nc.sync.dma_start(ps3[:, :, 0:496], pd3[:, :, 0:496])
            nc.sync.dma_start(ps3[:, :, 496:PS_COLS], pd3[:, :, 496:PS_COLS])
            nc.sync.dma_start(
                pq_sb[:].rearrange("p (d c) -> p d c", c=PQ_COLS),
                pq_d[:].rearrange("(d p) c -> p d c", p=128),
            )
            nc.sync.dma_start(a_sb[:], a_d[:])
            wload(1)

            def conv(cc, side):
                """Conv for channel chunk cc of one side -> conv_sb slab."""
                c0, ccw = CC0[cc], CCW[cc]
                src, dst, poschunks, cols, slab = (
                    (ps_sb, cs_sb, POSCH_S, PS_COLS, SLAB_S)
                    if side == 0
                    else (pq_sb, cq_sb, POSCH_Q, PQ_COLS, SLAB_Q)
                )
                for pos0, pw in poschunks:
                    psum = convps.tile([128, 488], f32, tag="conv")
                    mms = []
                    for d in range(4):
                        # first and last matmul of the accumulation group must
                        # cover the full partition range (start/stop semantics
                        # are per-element), so full-size delta groups bracket
                        order = DORDER if d < 3 else [1, 0, 3, 4, 2]
                        for di in order:
                            delta, sz = DELTAS[di]
                            if sz <= c0:
                                continue
                            wcc = min(ccw, sz - c0)
                            mms.append((d, di, delta, wcc))
                    for idx, (d, di, delta, wcc) in enumerate(mms):
                        lcol = d * 2 * WSIDE + WCOL[(cc, side, di)]
                        rcol = d * cols + pos0 + delta + 2
                        nc.tensor.matmul(
                            psum[0:wcc, 0:pw],
                            lhsT=w_sb[:, lcol : lcol + wcc],
                            rhs=src[:, rcol : rcol + pw],
                            start=(idx == 0),
                            stop=(idx == len(mms) - 1),
                        )
                    nc.scalar.copy(
                        dst[0:ccw, cc * slab + pos0 : cc * slab + pos0 + pw],
                        psum[0:ccw, 0:pw],
                    )

            def xevict(cc):
                """conv_sb -> DRAM staging in X[row, slot*31 + l] layout.

                One DMA per side: DRAM write APs have no partition-dim
                ordering constraint, so (p, r, l) iteration can scatter to
                row-major X. Keeps total DMA count (and per-DMA HWDGE fixed
                cost) low.
                """
                xc0 = cc * 128 * 31
                ccw = CCW[cc]
                nc.sync.dma_start(
                    bass.AP(
                        x_dram[:].tensor,
                        xc0,
                        [[31, ccw], [5 * 128 * 31, NROW], [1, 31]],
                    ),
                    bass.AP(
                        cs_sb[:].tensor,
                        cs_sb[:].offset + cc * SLAB_S + 2,
                        [[cs_sb[:].ap[0][0], ccw], [ROWSTR, NROW], [1, 31]],
                    ),
                )
                nc.sync.dma_start(
                    bass.AP(
                        x_dram[:].tensor,
                        NROW * 5 * 128 * 31 + xc0,
                        [[31, ccw], [5 * 128 * 31, JN], [1, 31]],
                    ),
                    bass.AP(
                        cq_sb[:].tensor,
                        cq_sb[:].offset + cc * SLAB_Q + 2,
                        [[cq_sb[:].ap[0][0], ccw], [ROWSTR, JN], [1, 31]],
                    ),
                )

            def xload(cc):
                """DRAM staging -> X tile [39, 3968] (contiguous rows)."""
                xc0 = cc * 128 * 31
                w = CCW[cc] * 31
                xt = xpool.tile([XROWS, 128 * 31], f32r, tag="x")
                nc.sync.dma_start(
                    xt[0 : XROWS - 1, 0:w],
                    x_dram[:, xc0 : xc0 + w],
                )
                nc.sync.dma_start(
                    xt[XROWS - 1 : XROWS, 0:w],
                    bias_d[0:1, xc0 : xc0 + w],
                )
                return xt

            def pairwise(cc, xt, reds):
                subs, groups = _sub_plan(cc)
                for mi, (moff, msz) in enumerate(MCH):
                    for g in groups:
                        pw = pwps.tile([109, 3, 512], f32, tag="pw")
                        for j, si in enumerate(g):
                            soff, sw = subs[si]
                            nc.tensor.matmul(
                                pw[0:msz, j : j + 1, 0:sw],
                                lhsT=a_sb[:, moff : moff + msz],
                                rhs=xt[:, soff : soff + sw],
                                start=True,
                                stop=True,
                            )
                        ng = len(g)
                        gsw = subs[g[0]][1]
                        ncols = ng * (gsw // 31)
                        cb = CC0[cc] + subs[g[0]][0] // 31
                        nc.vector.tensor_reduce(
                            reds[mi][0:msz, cb : cb + ncols],
                            pw[0:msz, 0:ng, 0:gsw].rearrange(
                                "p g (c l) -> p g c l", l=31
                            ),
                            axis=mybir.AxisListType.X,
                            op=mybir.AluOpType.max,
                        )

            reds = [
                redpool.tile([109, NCH], f32, tag="red", name=f"red{i}")
                for i in range(3)
            ]

            # software-pipelined emission: conv leads xbuild by 1 chunk,
            # pairwise lags conv by 2 chunks (keeps PE fed while X DMAs land)
            xts = {}
            conv(0, 0)
            conv(0, 1)
            xevict(0)
            xts[0] = xload(0)
            for cc in range(1, 5):
                if cc + 1 <= 4:
                    wload(cc + 1)
                conv(cc, 0)
                conv(cc, 1)
                xevict(cc)
                xts[cc] = xload(cc)
                pairwise(cc - 1, xts.pop(cc - 1), reds)
            pairwise(4, xts.pop(4), reds)

            for mi, (moff, msz) in enumerate(MCH):
                nc.scalar.activation(
                    reds[mi][0:msz, :],
                    reds[mi][0:msz, :],
                    mybir.ActivationFunctionType.Relu,
                )
                nc.sync.dma_start(
                    out_d[moff : moff + msz, :], reds[mi][0:msz, 0:NCH]
                )

    nc.compile()
    return nc


def get_program():
    global _PROG
    if _PROG is None:
        _PROG = _build_program()
    return _PROG


def build_inputs(s, q, ws, bs):
    """Host-side shard prep. ws/bs: dicts k -> w(150, 1024, k) / b(150,).

    Returns (in_maps, core_meta). Core c handles episode c//2, q-row half c%2.
    """
    s = np.asarray(s, dtype=np.float32).reshape(B, NROW, L, D)
    q = np.asarray(q, dtype=np.float32).reshape(B, NQROW, L, D)

    # packed weights [D, 2*2100]: per side, delta groups at WOFF offsets,
    # device channel order [k5|k4|k3|k2]
    wall = np.zeros((D, 2 * WSIDE), dtype=np.float32)
    bias_dev = np.zeros(NCH, dtype=np.float32)
    for k in (2, 3, 4, 5):
        blk = ORD_OF_K[k] * 150
        bias_dev[blk : blk + 150] = bs[k]
        for di, (delta, sz) in enumerate(DELTAS):
            t = delta + PAD_OF_K[k]
            if not (0 <= t < k):
                continue
            assert blk + 150 <= sz
            wall[:, WOFF[di] + blk : WOFF[di] + blk + 150] = ws[k][:, :D, t].T
            wall[:, WSIDE + WOFF[di] + blk : WSIDE + WOFF[di] + blk + 150] = (
                ws[k][:, D:, t].T
            )
    perm = np.zeros(2 * WSIDE, dtype=np.int64)
    for side in range(2):
        for di, (_, sz) in enumerate(DELTAS):
            for cc in range(5):
                c0 = CC0[cc]
                if sz <= c0:
                    continue
                w = min(128, sz - c0)
                newc = WCOL[(cc, side, di)]
                oldc = side * WSIDE + WOFF[di] + c0
                perm[newc : newc + w] = np.arange(oldc, oldc + w)
    wall = wall[:, perm]

    bias_pad = np.zeros(5 * 128 * 31, dtype=np.float32)
    bias_pad[: NCH * 31] = np.repeat(bias_dev, 31)
    bias_rep = bias_pad[None, :]

    amat = np.zeros((XROWS, NPAIR), dtype=np.float32)
    for i in range(NROW):
        for t in range(JN):
            p = i * JN + t
            amat[i, p] = 1.0
            amat[NROW + t, p] = 1.0
    amat[XROWS - 1, :] = 1.0

    in_maps = []
    for core in range(8):
        b, jh = core // 2, core % 2
        jidx = [min(jh * JN + t, NQROW - 1) for t in range(JN)]
        psa = np.zeros((D, PS_COLS), dtype=np.float32)
        pqa = np.zeros((D, PQ_COLS), dtype=np.float32)
        for r in range(NROW):
            psa[:, r * ROWSTR + 4 : r * ROWSTR + 4 + L] = s[b, r].T
        for t, j in enumerate(jidx):
            pqa[:, t * ROWSTR + 4 : t * ROWSTR + 4 + L] = q[b, j].T
        in_maps.append(
            {"ps": psa, "pq": pqa, "w": wall, "a": amat, "bias": bias_rep}
        )
    return in_maps


# device channel -> original output channel maps
_S_IDX = np.array(
    [(3 - g) * 150 + u for g in range(4) for u in range(75)], dtype=np.int64
)
_Q_IDX = _S_IDX + 75


def assemble_outputs(core_outs):
    """core_outs: list of 8 arrays [NPAIR, NCH] -> (s_out, q_out)."""
    s_out = np.empty((B, NROW, NQROW, 300), dtype=np.float32)
    q_out = np.empty((B, NROW, NQROW, 300), dtype=np.float32)
    for core in range(8):
        b, jh = core // 2, core % 2
        nj = JN if jh == 0 else NQROW - JN
        arr = np.ascontiguousarray(core_outs[core]).reshape(NROW, JN, NCH)
        s_out[b, :, jh * JN : jh * JN + nj] = arr[:, :nj][:, :, _S_IDX]
        q_out[b, :, jh * JN : jh * JN + nj] = arr[:, :nj][:, :, _Q_IDX]
    return s_out.reshape(-1, 300), q_out.reshape(-1, 300)


def kernel(s, q, w2, b2, w3, b3, w4, b4, w5, b5, B=4, N=5, K=5, Q=5, L=31):
    ws = {2: np.asarray(w2, np.float32), 3: np.asarray(w3, np.float32),
          4: np.asarray(w4, np.float32), 5: np.asarray(w5, np.float32)}
    bs = {2: np.asarray(b2, np.float32), 3: np.asarray(b3, np.float32),
          4: np.asarray(b4, np.float32), 5: np.asarray(b5, np.float32)}
    in_maps = build_inputs(s, q, ws, bs)
    nc = get_program()
    res = run_bass_kernel_spmd(nc, in_maps, list(range(8))).results
    return assemble_outputs([res[c]["out"] for c in range(8)])



# revision 5
# speedup vs baseline: 1.3843x; 1.3843x over previous
"""Trainium2 Bass kernel for nn_CNNInteractLayer (CNN interaction layer).

Math: for each episode b, s-row i, q-row j:
  out[b,i,j] = maxpool_L(relu(conv_k(concat(s[b,i], q[b,j])))) for k in 2..5
Factorization: conv(concat(s,q)) = conv_s(s) + conv_q(q) + bias, so per-row
convolutions run once on the PE (bf16, fp32 psum), and the pairwise stage is
fused on the vector engines: the scalar engine evicts conv psum to SBUF bf16
(s-side with bias folded in), the DVE forms pairwise sums with a broadcast
tensor_tensor add (2x bf16 mode) plus the first max-tree level, and the
otherwise-idle GPSIMD engine finishes the max tree. No pairwise matmul, no
A-matrix, no DRAM transpose roundtrip.

Sharding: 8 cores = 4 episodes x 2 halves of the q-row range.
"""

import os
import sys

import numpy as np

for _p in ("/opt/trn_rl_repo",):
    if os.path.isdir(_p) and _p not in sys.path:
        sys.path.insert(0, _p)

# the bass runner needs the axon jax backend; don't let a cpu-only pin hide it
if "axon" not in os.environ.get("JAX_PLATFORMS", "axon"):
    os.environ.pop("JAX_PLATFORMS", None)

import ml_dtypes  # noqa: E402

from concourse import bacc, bass, mybir, tile  # noqa: E402
from concourse.bass_utils import run_bass_kernel_spmd  # noqa: E402

BF16 = np.dtype(ml_dtypes.bfloat16)

# Problem dims (hardcoded per spec)
B, N, K, Q, L, D = 4, 5, 5, 5, 31, 512
NROW = N * K            # 25 s-rows per episode
NQROW = N * Q           # 25 q-rows per episode
JN = 13                 # q-rows per core (padded; odd cores use 12)
ROWSTR = L + 2          # 33: 2-zero gap between rows gives conv zero-padding
PS_COLS = NROW * ROWSTR + 9   # 834 (row r data at r*33+4 .. +34; halo right)
PQ_COLS = JN * ROWSTR + 9     # 438
S_OUT = 826             # conv output positions computed, s side (even)
Q_OUT = 430             # q side (even)
SLAB_S = 828            # per-chunk slab stride in cs_sb
SLAB_Q = 432
NCH = 600               # device channels: [k5 | k4 | k3 | k2] x 150
# delta (tap shift) groups; prefix-size in device channel order
DELTAS = [(-2, 300), (-1, 600), (0, 600), (1, 450), (2, 150)]
# emission order per d-chunk: full-coverage groups first so the first matmul
# of each PSUM accumulation group writes the full partition range
DORDER = [1, 2, 0, 3, 4]
WOFF = [0, 300, 900, 1500, 1950]  # packed col offset of each delta group
WSIDE = 2100
CC0 = [0, 128, 256, 384, 512]     # channel chunk starts
CCW = [128, 128, 128, 128, 88]
NPAIR = NROW * JN                 # 325
PAD_OF_K = {2: 1, 3: 1, 4: 2, 5: 2}
ORD_OF_K = {5: 0, 4: 1, 3: 2, 2: 3}
POSCH_S = [(0, 416), (416, 410)]
POSCH_Q = [(0, 430)]
# conv emission order: smallest chunks first to fill the DVE/Pool pipe early
CCORDER = [4, 3, 2, 1, 0]
JBLOCKS = [(0, 5), (5, 4), (9, 4)]  # pairwise j-blocks

# chunk-major packed-W layout: per channel chunk, [side s | side q], each a
# concatenation of the valid delta groups' column slices for that chunk
def _chunk_tables():
    chw = []          # per-side width of each chunk block
    coloff = {}       # (cc, side, di) -> column offset in packed W
    off = 0
    for cc in range(5):
        c0 = CC0[cc]
        widths = []
        for di, (_, sz) in enumerate(DELTAS):
            w = min(128, sz - c0) if sz > c0 else 0
            widths.append(w)
        side_w = sum(widths)
        for side in range(2):
            p = off + side * side_w
            for di, w in enumerate(widths):
                if w:
                    coloff[(cc, side, di)] = p
                    p += w
        chw.append(side_w)
        off += 2 * side_w
    return chw, coloff


CHW, WCOL = _chunk_tables()
CHOFF = [sum(2 * w for w in CHW[:i]) for i in range(6)]

_PROG = None


def _build_program():
    nc = bacc.Bacc("TRN2", target_bir_lowering=False, debug=False, num_devices=8)
    f32 = mybir.dt.float32
    bf16 = mybir.dt.bfloat16

    ps_d = nc.dram_tensor("ps", [D, PS_COLS], bf16, kind="ExternalInput")
    pq_d = nc.dram_tensor("pq", [D, PQ_COLS], bf16, kind="ExternalInput")
    w_d = nc.dram_tensor("w", [D, 2 * WSIDE], bf16, kind="ExternalInput")
    bias_d = nc.dram_tensor("bias", [640, 1], f32, kind="ExternalInput")
    out_d = nc.dram_tensor("out", [NCH, NPAIR], bf16, kind="ExternalOutput")

    with tile.TileContext(nc) as tc:
        with (
            tc.tile_pool(name="persist", bufs=1) as big,
            tc.tile_pool(name="tmppool", bufs=3) as tmppool,
            tc.tile_pool(name="t16pool", bufs=3) as t16pool,
            tc.tile_pool(name="t8pool", bufs=2) as t8pool,
            tc.tile_pool(name="t4pool", bufs=2) as t4pool,
            tc.tile_pool(name="t2pool", bufs=2) as t2pool,
            tc.tile_pool(name="convps", bufs=3, space="PSUM") as convps,
        ):
            w_sb = big.tile([128, 4 * 2 * WSIDE], bf16, tag="w")
            ps_sb = big.tile([128, 4 * PS_COLS], bf16, tag="ps")
            pq_sb = big.tile([128, 4 * PQ_COLS], bf16, tag="pq")
            cs_sb = big.tile([128, 5 * SLAB_S], bf16, tag="cs")
            cq_sb = big.tile([128, 5 * SLAB_Q], bf16, tag="cq")
            bias_sb = big.tile([128, 6], f32, tag="bias")
            red = big.tile([128, 5 * NPAIR], bf16, tag="red")

            def wload(cc):
                wd = w_d[:].rearrange("(d p) c -> p d c", p=128)
                ws = w_sb[:].rearrange("p (d c) -> p d c", c=2 * WSIDE)
                nc.sync.dma_start(
                    ws[:, :, CHOFF[cc] : CHOFF[cc + 1]],
                    wd[:, :, CHOFF[cc] : CHOFF[cc + 1]],
                )

            wload(CCORDER[0])
            nc.sync.dma_start(
                ps_sb[:].rearrange("p (d c) -> p d c", c=PS_COLS),
                ps_d[:].rearrange("(d p) c -> p d c", p=128),
            )
            nc.sync.dma_start(
                pq_sb[:].rearrange("p (d c) -> p d c", c=PQ_COLS),
                pq_d[:].rearrange("(d p) c -> p d c", p=128),
            )
            # bias[c] at dram row c -> bias_sb[p, cc] for c = cc*128+p
            nc.sync.dma_start(
                bias_sb[:, 0:5],
                bass.AP(bias_d[:].tensor, 0, [[1, 128], [128, 5]]),
            )
            nc.vector.memset(bias_sb[:, 5:6], 0.0)
            wload(CCORDER[1])

            def conv(cc, side):
                """Conv for channel chunk cc of one side -> cs/cq slab."""
                c0, ccw = CC0[cc], CCW[cc]
                src, dst, poschunks, cols, slab, bcol = (
                    (ps_sb, cs_sb, POSCH_S, PS_COLS, SLAB_S, cc)
                    if side == 0
                    else (pq_sb, cq_sb, POSCH_Q, PQ_COLS, SLAB_Q, 5)
                )
                src3 = src[:].rearrange("p (d c) -> p d c", c=cols)
                for pos0, pw in poschunks:
                    psum = convps.tile([128, 432], f32, tag="conv")
                    mms = []
                    for d in range(4):
                        # first and last matmul of the accumulation group must
                        # cover the full partition range (start/stop semantics
                        # are per-element), so full-size delta groups bracket
                        order = DORDER if d < 3 else [1, 0, 3, 4, 2]
                        for di in order:
                            delta, sz = DELTAS[di]
                            if sz <= c0:
                                continue
                            wcc = min(ccw, sz - c0)
                            mms.append((d, di, delta, wcc))
                    for idx, (d, di, delta, wcc) in enumerate(mms):
                        lcol = d * 2 * WSIDE + WCOL[(cc, side, di)]
                        rcol = pos0 + delta + 2
                        nc.tensor.matmul(
                            psum[0:wcc, 0:pw],
                            lhsT=w_sb[:, lcol : lcol + wcc],
                            rhs=src3[:, d, rcol : rcol + pw],
                            start=(idx == 0),
                            stop=(idx == len(mms) - 1),
                        )
                    # psum -> SBUF bf16 on the scalar engine; bias folds into
                    # the s side so each pairwise sum gets it exactly once
                    nc.scalar.add(
                        dst[0:ccw, cc * slab + pos0 : cc * slab + pos0 + pw],
                        psum[0:ccw, 0:pw],
                        bias_sb[0:ccw, bcol : bcol + 1],
                    )

            def pairwise(cc):
                """Fused pairwise add + maxpool for chunk cc.

                tmp[ch,j,i,l] = cs[ch,i,l] + cq[ch,j,l]   (DVE, 2x bf16)
                L1 16-wide max on DVE, L2..L5 on GPSIMD, relu on DVE.
                """
                ccw = CCW[cc]
                mx = mybir.AluOpType.max
                for j0, jb in JBLOCKS:
                    npr = jb * NROW
                    tmp = tmppool.tile([128, 5 * NROW * 31], bf16, tag="tmp")
                    t16 = t16pool.tile([128, 5 * NROW * 16], bf16, tag="t16")
                    t8 = t8pool.tile([128, 5 * NROW * 8], bf16, tag="t8")
                    t4 = t4pool.tile([128, 5 * NROW * 4], bf16, tag="t4")
                    t2 = t2pool.tile([128, 5 * NROW * 2], bf16, tag="t2")
                    def ap(t, off, dims):
                        tap = t[:]
                        return bass.AP(
                            tap.tensor,
                            tap.offset + off,
                            [[tap.ap[0][0], ccw]] + dims,
                        )

                    nc.vector.tensor_tensor(
                        ap(tmp, 0, [[775, jb], [31, NROW], [1, 31]]),
                        ap(cs_sb, cc * SLAB_S + 2,
                           [[0, jb], [ROWSTR, NROW], [1, 31]]),
                        ap(cq_sb, cc * SLAB_Q + 2 + j0 * ROWSTR,
                           [[ROWSTR, jb], [0, NROW], [1, 31]]),
                        op=mybir.AluOpType.add,
                    )
                    nc.vector.tensor_tensor(
                        ap(t16, 0, [[16, npr], [1, 16]]),
                        ap(tmp, 0, [[31, npr], [1, 16]]),
                        ap(tmp, 15, [[31, npr], [1, 16]]),
                        op=mx,
                    )
                    nc.vector.tensor_tensor(
                        ap(t8, 0, [[8, npr], [1, 8]]),
                        ap(t16, 0, [[16, npr], [1, 8]]),
                        ap(t16, 8, [[16, npr], [1, 8]]),
                        op=mx,
                    )
                    nc.vector.tensor_tensor(
                        ap(t4, 0, [[4, npr], [1, 4]]),
                        ap(t8, 0, [[8, npr], [1, 4]]),
                        ap(t8, 4, [[8, npr], [1, 4]]),
                        op=mx,
                    )
                    nc.vector.tensor_tensor(
                        ap(t2, 0, [[2, npr], [1, 2]]),
                        ap(t4, 0, [[4, npr], [1, 2]]),
                        ap(t4, 2, [[4, npr], [1, 2]]),
                        op=mx,
                    )
                    nc.vector.tensor_tensor(
                        ap(red, cc * NPAIR + j0 * NROW, [[1, npr]]),
                        ap(t2, 0, [[2, npr]]),
                        ap(t2, 1, [[2, npr]]),
                        op=mx,
                    )
                # relu on the scalar engine (DVE is the bottleneck)
                nc.scalar.activation(
                    red[0:ccw, cc * NPAIR : (cc + 1) * NPAIR],
                    red[0:ccw, cc * NPAIR : (cc + 1) * NPAIR],
                    mybir.ActivationFunctionType.Relu,
                )
                nc.sync.dma_start(
                    out_d[CC0[cc] : CC0[cc] + ccw, :],
                    red[0:ccw, cc * NPAIR : (cc + 1) * NPAIR],
                )

            # software pipeline: conv leads pairwise by one chunk
            conv(CCORDER[0], 0)
            conv(CCORDER[0], 1)
            for k in range(1, 5):
                if k + 1 <= 4:
                    wload(CCORDER[k + 1])
                conv(CCORDER[k], 0)
                conv(CCORDER[k], 1)
                pairwise(CCORDER[k - 1])
            pairwise(CCORDER[4])

    nc.compile()
    return nc


def get_program():
    global _PROG
    if _PROG is None:
        _PROG = _build_program()
    return _PROG


def build_inputs(s, q, ws, bs):
    """Host-side shard prep. ws/bs: dicts k -> w(150, 1024, k) / b(150,).

    Returns in_maps. Core c handles episode c//2, q-row half c%2.
    """
    s = np.asarray(s, dtype=np.float32).reshape(B, NROW, L, D)
    q = np.asarray(q, dtype=np.float32).reshape(B, NQROW, L, D)

    # packed weights [D, 2*2100]: per side, delta groups at WOFF offsets,
    # device channel order [k5|k4|k3|k2]
    wall = np.zeros((D, 2 * WSIDE), dtype=np.float32)
    bias_dev = np.zeros(640, dtype=np.float32)
    for k in (2, 3, 4, 5):
        blk = ORD_OF_K[k] * 150
        bias_dev[blk : blk + 150] = bs[k]
        for di, (delta, sz) in enumerate(DELTAS):
            t = delta + PAD_OF_K[k]
            if not (0 <= t < k):
                continue
            assert blk + 150 <= sz
            wall[:, WOFF[di] + blk : WOFF[di] + blk + 150] = ws[k][:, :D, t].T
            wall[:, WSIDE + WOFF[di] + blk : WSIDE + WOFF[di] + blk + 150] = (
                ws[k][:, D:, t].T
            )
    perm = np.zeros(2 * WSIDE, dtype=np.int64)
    for side in range(2):
        for di, (_, sz) in enumerate(DELTAS):
            for cc in range(5):
                c0 = CC0[cc]
                if sz <= c0:
                    continue
                w = min(128, sz - c0)
                newc = WCOL[(cc, side, di)]
                oldc = side * WSIDE + WOFF[di] + c0
                perm[newc : newc + w] = np.arange(oldc, oldc + w)
    wall = wall[:, perm].astype(BF16)
    bias_col = bias_dev[:, None]

    in_maps = []
    for core in range(8):
        b, jh = core // 2, core % 2
        jidx = [min(jh * JN + t, NQROW - 1) for t in range(JN)]
        psa = np.zeros((D, PS_COLS), dtype=np.float32)
        pqa = np.zeros((D, PQ_COLS), dtype=np.float32)
        for r in range(NROW):
            psa[:, r * ROWSTR + 4 : r * ROWSTR + 4 + L] = s[b, r].T
        for t, j in enumerate(jidx):
            pqa[:, t * ROWSTR + 4 : t * ROWSTR + 4 + L] = q[b, j].T
        in_maps.append(
            {
                "ps": psa.astype(BF16),
                "pq": pqa.astype(BF16),
                "w": wall,
                "bias": bias_col,
            }
        )
    return in_maps


# device channel -> original output channel maps
_S_IDX = np.array(
    [(3 - g) * 150 + u for g in range(4) for u in range(75)], dtype=np.int64
)
_Q_IDX = _S_IDX + 75


def assemble_outputs(core_outs):
    """core_outs: list of 8 arrays [NCH, NPAIR] -> (s_out, q_out)."""
    s_out = np.empty((B, NROW, NQROW, 300), dtype=np.float32)
    q_out = np.empty((B, NROW, NQROW, 300), dtype=np.float32)
    for core in range(8):
        b, jh = core // 2, core % 2
        nj = JN if jh == 0 else NQROW - JN
        # out[ch, j*25+i] -> [j, i, ch]
        arr = (
            np.asarray(core_outs[core])
            .astype(np.float32)
            .reshape(NCH, JN, NROW)
            .transpose(1, 2, 0)
        )
        s_out[b, :, jh * JN : jh * JN + nj] = arr[:nj][:, :, _S_IDX].transpose(
            1, 0, 2
        )
        q_out[b, :, jh * JN : jh * JN + nj] = arr[:nj][:, :, _Q_IDX].transpose(
            1, 0, 2
        )
    return s_out.reshape(-1, 300), q_out.reshape(-1, 300)


def kernel(s, q, w2, b2, w3, b3, w4, b4, w5, b5, B=4, N=5, K=5, Q=5, L=31):
    ws = {2: np.asarray(w2, np.float32), 3: np.asarray(w3, np.float32),
          4: np.asarray(w4, np.float32), 5: np.asarray(w5, np.float32)}
    bs = {2: np.asarray(b2, np.float32), 3: np.asarray(b3, np.float32),
          4: np.asarray(b4, np.float32), 5: np.asarray(b5, np.float32)}
    in_maps = build_inputs(s, q, ws, bs)
    nc = get_program()
    res = run_bass_kernel_spmd(nc, in_maps, list(range(8))).results
    return assemble_outputs([res[c]["out"] for c in range(8)])


# revision 7
# speedup vs baseline: 1.4866x; 1.0739x over previous
"""Trainium2 Bass kernel for nn_CNNInteractLayer (CNN interaction layer).

Math: for each episode b, s-row i, q-row j:
  out[b,i,j] = maxpool_L(relu(conv_k(concat(s[b,i], q[b,j])))) for k in 2..5
Factorization: conv(concat(s,q)) = conv_s(s) + conv_q(q) + bias, so per-row
convolutions run once on the PE (bf16, fp32 psum), and the pairwise stage is
fused on the vector engines: the scalar engine evicts conv psum to SBUF bf16
(s-side with bias folded in), the DVE forms pairwise sums with a broadcast
tensor_tensor add (2x bf16 mode) plus the first max-tree level, and the
otherwise-idle GPSIMD engine finishes the max tree. No pairwise matmul, no
A-matrix, no DRAM transpose roundtrip.

Sharding: 8 cores = 4 episodes x 2 halves of the q-row range.
"""

import os
import sys

import numpy as np

for _p in ("/opt/trn_rl_repo",):
    if os.path.isdir(_p) and _p not in sys.path:
        sys.path.insert(0, _p)

# the bass runner needs the axon jax backend; don't let a cpu-only pin hide it
if "axon" not in os.environ.get("JAX_PLATFORMS", "axon"):
    os.environ.pop("JAX_PLATFORMS", None)

import ml_dtypes  # noqa: E402

from concourse import bacc, bass, mybir, tile  # noqa: E402
from concourse.bass_utils import run_bass_kernel_spmd  # noqa: E402

BF16 = np.dtype(ml_dtypes.bfloat16)

# Problem dims (hardcoded per spec)
B, N, K, Q, L, D = 4, 5, 5, 5, 31, 512
NROW = N * K            # 25 s-rows per episode
NQROW = N * Q           # 25 q-rows per episode
JN = 13                 # q-rows per core (padded; odd cores use 12)
ROWSTR = L + 2          # 33: 2-zero gap between rows gives conv zero-padding
PS_COLS = NROW * ROWSTR + 9   # 834 (row r data at r*33+4 .. +34; halo right)
PQ_COLS = JN * ROWSTR + 9     # 438
S_OUT = 826             # conv output positions computed, s side (even)
Q_OUT = 430             # q side (even)
SLAB_S = 828            # per-chunk slab stride in cs_sb
SLAB_Q = 432
NCH = 600               # device channels: [k5 | k4 | k3 | k2] x 150
# delta (tap shift) groups; prefix-size in device channel order
DELTAS = [(-2, 300), (-1, 600), (0, 600), (1, 450), (2, 150)]
# emission order per d-chunk: full-coverage groups first so the first matmul
# of each PSUM accumulation group writes the full partition range
DORDER = [1, 2, 0, 3, 4]
WOFF = [0, 300, 900, 1500, 1950]  # packed col offset of each delta group
WSIDE = 2100
CC0 = [0, 128, 256, 384, 512]     # channel chunk starts
CCW = [128, 128, 128, 128, 88]
NPAIR = NROW * JN                 # 325
PAD_OF_K = {2: 1, 3: 1, 4: 2, 5: 2}
ORD_OF_K = {5: 0, 4: 1, 3: 2, 2: 3}
POSCH_S = [(0, 416), (416, 410)]
POSCH_Q = [(0, 430)]
# conv emission order: smallest chunks first to fill the DVE/Pool pipe early
CCORDER = [4, 3, 2, 1, 0]
JBLOCKS = [(0, 13)]  # pairwise j-blocks (single: fewer DVE instr inits)

# chunk-major packed-W layout: per channel chunk, [side s | side q], each a
# concatenation of the valid delta groups' column slices for that chunk
def _chunk_tables():
    chw = []          # per-side width of each chunk block
    coloff = {}       # (cc, side, di) -> column offset in packed W
    off = 0
    for cc in range(5):
        c0 = CC0[cc]
        widths = []
        for di, (_, sz) in enumerate(DELTAS):
            w = min(128, sz - c0) if sz > c0 else 0
            widths.append(w)
        side_w = sum(widths)
        for side in range(2):
            p = off + side * side_w
            for di, w in enumerate(widths):
                if w:
                    coloff[(cc, side, di)] = p
                    p += w
        chw.append(side_w)
        off += 2 * side_w
    return chw, coloff


CHW, WCOL = _chunk_tables()
CHOFF = [sum(2 * w for w in CHW[:i]) for i in range(6)]

_PROG = None


def _build_program():
    nc = bacc.Bacc("TRN2", target_bir_lowering=False, debug=False, num_devices=8)
    f32 = mybir.dt.float32
    bf16 = mybir.dt.bfloat16

    ps_d = nc.dram_tensor("ps", [D, PS_COLS], bf16, kind="ExternalInput")
    pq_d = nc.dram_tensor("pq", [D, PQ_COLS], bf16, kind="ExternalInput")
    w_d = nc.dram_tensor("w", [D, 2 * WSIDE], bf16, kind="ExternalInput")
    bias_d = nc.dram_tensor("bias", [640, 1], f32, kind="ExternalInput")
    out_d = nc.dram_tensor("out", [NCH, NPAIR], bf16, kind="ExternalOutput")

    with tile.TileContext(nc) as tc:
        with (
            tc.tile_pool(name="persist", bufs=1) as big,
            tc.tile_pool(name="tmppool", bufs=2) as tmppool,
            tc.tile_pool(name="t16pool", bufs=2) as t16pool,
            tc.tile_pool(name="t8pool", bufs=2) as t8pool,
            tc.tile_pool(name="t4pool", bufs=2) as t4pool,
            tc.tile_pool(name="t2pool", bufs=2) as t2pool,
            tc.tile_pool(name="convps", bufs=3, space="PSUM") as convps,
        ):
            w_sb = big.tile([128, 4 * 2 * WSIDE], bf16, tag="w")
            ps_sb = big.tile([128, 4 * PS_COLS], bf16, tag="ps")
            pq_sb = big.tile([128, 4 * PQ_COLS], bf16, tag="pq")
            cs_sb = big.tile([128, 5 * SLAB_S], bf16, tag="cs")
            cq_sb = big.tile([128, 5 * SLAB_Q], bf16, tag="cq")
            bias_sb = big.tile([128, 6], f32, tag="bias")
            red = big.tile([128, 5 * NPAIR], bf16, tag="red")

            def wload(cc):
                wd = w_d[:].rearrange("(d p) c -> p d c", p=128)
                ws = w_sb[:].rearrange("p (d c) -> p d c", c=2 * WSIDE)
                nc.sync.dma_start(
                    ws[:, :, CHOFF[cc] : CHOFF[cc + 1]],
                    wd[:, :, CHOFF[cc] : CHOFF[cc + 1]],
                )

            wload(CCORDER[0])
            ps3 = ps_sb[:].rearrange("p (d c) -> p d c", c=PS_COLS)
            pd3 = ps_d[:].rearrange("(d p) c -> p d c", p=128)
            for d in range(4):
                nc.sync.dma_start(ps3[:, d : d + 1, :], pd3[:, d : d + 1, :])
            pq3 = pq_sb[:].rearrange("p (d c) -> p d c", c=PQ_COLS)
            qd3 = pq_d[:].rearrange("(d p) c -> p d c", p=128)
            for d in range(4):
                nc.sync.dma_start(pq3[:, d : d + 1, :], qd3[:, d : d + 1, :])
            # bias[c] at dram row c -> bias_sb[p, cc] for c = cc*128+p
            nc.sync.dma_start(
                bias_sb[:, 0:5],
                bass.AP(bias_d[:].tensor, 0, [[1, 128], [128, 5]]),
            )
            nc.vector.memset(bias_sb[:, 5:6], 0.0)
            # touch the activation table during the DMA prologue so the
            # 1.3us LoadActFuncSet is off the critical path
            nc.scalar.activation(
                bias_sb[:, 5:6], bias_sb[:, 5:6],
                mybir.ActivationFunctionType.Relu,
            )
            wload(CCORDER[1])

            def conv(cc, side):
                """Conv for channel chunk cc of one side -> cs/cq slab."""
                c0, ccw = CC0[cc], CCW[cc]
                src, dst, poschunks, cols, slab, bcol = (
                    (ps_sb, cs_sb, POSCH_S, PS_COLS, SLAB_S, cc)
                    if side == 0
                    else (pq_sb, cq_sb, POSCH_Q, PQ_COLS, SLAB_Q, 5)
                )
                src3 = src[:].rearrange("p (d c) -> p d c", c=cols)
                for pos0, pw in poschunks:
                    psum = convps.tile([128, 432], f32, tag="conv")
                    mms = []
                    for d in range(4):
                        # first and last matmul of the accumulation group must
                        # cover the full partition range (start/stop semantics
                        # are per-element), so full-size delta groups bracket
                        order = DORDER if d < 3 else [1, 0, 3, 4, 2]
                        for di in order:
                            delta, sz = DELTAS[di]
                            if sz <= c0:
                                continue
                            wcc = min(ccw, sz - c0)
                            mms.append((d, di, delta, wcc))
                    for idx, (d, di, delta, wcc) in enumerate(mms):
                        lcol = d * 2 * WSIDE + WCOL[(cc, side, di)]
                        rcol = pos0 + delta + 2
                        nc.tensor.matmul(
                            psum[0:wcc, 0:pw],
                            lhsT=w_sb[:, lcol : lcol + wcc],
                            rhs=src3[:, d, rcol : rcol + pw],
                            start=(idx == 0),
                            stop=(idx == len(mms) - 1),
                        )
                    # psum -> SBUF bf16 on the scalar engine; bias folds into
                    # the s side so each pairwise sum gets it exactly once
                    nc.scalar.add(
                        dst[0:ccw, cc * slab + pos0 : cc * slab + pos0 + pw],
                        psum[0:ccw, 0:pw],
                        bias_sb[0:ccw, bcol : bcol + 1],
                    )

            def pairwise(cc, last=False):
                """Fused pairwise add + maxpool for chunk cc.

                tmp[ch,j,i,l] = cs[ch,i,l] + cq[ch,j,l]   (DVE, 2x bf16)
                L1 16-wide max on DVE, L2..L5 on GPSIMD, relu on DVE.
                """
                ccw = CCW[cc]
                mx = mybir.AluOpType.max
                for j0, jb in JBLOCKS:
                    npr = jb * NROW
                    tmp = tmppool.tile([128, JN * NROW * 31], bf16, tag="tmp")
                    t16 = t16pool.tile([128, JN * NROW * 16], bf16, tag="t16")
                    t8 = t8pool.tile([128, JN * NROW * 8], bf16, tag="t8")
                    t4 = t4pool.tile([128, JN * NROW * 4], bf16, tag="t4")
                    t2 = t2pool.tile([128, JN * NROW * 2], bf16, tag="t2")
                    def ap(t, off, dims):
                        tap = t[:]
                        return bass.AP(
                            tap.tensor,
                            tap.offset + off,
                            [[tap.ap[0][0], ccw]] + dims,
                        )

                    nc.vector.tensor_tensor(
                        ap(tmp, 0, [[775, jb], [31, NROW], [1, 31]]),
                        ap(cs_sb, cc * SLAB_S + 2,
                           [[0, jb], [ROWSTR, NROW], [1, 31]]),
                        ap(cq_sb, cc * SLAB_Q + 2 + j0 * ROWSTR,
                           [[ROWSTR, jb], [0, NROW], [1, 31]]),
                        op=mybir.AluOpType.add,
                    )
                    nc.vector.tensor_tensor(
                        ap(t16, 0, [[16, npr], [1, 16]]),
                        ap(tmp, 0, [[31, npr], [1, 16]]),
                        ap(tmp, 15, [[31, npr], [1, 16]]),
                        op=mx,
                    )
                    nc.vector.tensor_tensor(
                        ap(t8, 0, [[8, npr], [1, 8]]),
                        ap(t16, 0, [[16, npr], [1, 8]]),
                        ap(t16, 8, [[16, npr], [1, 8]]),
                        op=mx,
                    )
                    nc.vector.tensor_tensor(
                        ap(t4, 0, [[4, npr], [1, 4]]),
                        ap(t8, 0, [[8, npr], [1, 4]]),
                        ap(t8, 4, [[8, npr], [1, 4]]),
                        op=mx,
                    )
                    nc.vector.tensor_tensor(
                        ap(t2, 0, [[2, npr], [1, 2]]),
                        ap(t4, 0, [[4, npr], [1, 2]]),
                        ap(t4, 2, [[4, npr], [1, 2]]),
                        op=mx,
                    )
                    nc.vector.tensor_tensor(
                        ap(red, cc * NPAIR + j0 * NROW, [[1, npr]]),
                        ap(t2, 0, [[2, npr]]),
                        ap(t2, 1, [[2, npr]]),
                        op=mx,
                    )
                # relu: scalar engine usually; DVE for the last chunk so
                # the tail is not gated on the Act pipeline
                if last:
                    nc.vector.tensor_scalar_max(
                        red[0:ccw, cc * NPAIR : (cc + 1) * NPAIR],
                        red[0:ccw, cc * NPAIR : (cc + 1) * NPAIR],
                        0.0,
                    )
                else:
                    nc.scalar.activation(
                        red[0:ccw, cc * NPAIR : (cc + 1) * NPAIR],
                        red[0:ccw, cc * NPAIR : (cc + 1) * NPAIR],
                        mybir.ActivationFunctionType.Relu,
                    )
                nc.sync.dma_start(
                    out_d[CC0[cc] : CC0[cc] + ccw, :],
                    red[0:ccw, cc * NPAIR : (cc + 1) * NPAIR],
                )

            # software pipeline: conv leads pairwise by one chunk
            conv(CCORDER[0], 0)
            conv(CCORDER[0], 1)
            for k in range(1, 5):
                if k + 1 <= 4:
                    wload(CCORDER[k + 1])
                conv(CCORDER[k], 0)
                conv(CCORDER[k], 1)
                pairwise(CCORDER[k - 1])
            pairwise(CCORDER[4], last=True)

    nc.compile()
    return nc


def get_program():
    global _PROG
    if _PROG is None:
        _PROG = _build_program()
    return _PROG


def build_inputs(s, q, ws, bs):
    """Host-side shard prep. ws/bs: dicts k -> w(150, 1024, k) / b(150,).

    Returns in_maps. Core c handles episode c//2, q-row half c%2.
    """
    s = np.asarray(s, dtype=np.float32).reshape(B, NROW, L, D)
    q = np.asarray(q, dtype=np.float32).reshape(B, NQROW, L, D)

    # packed weights [D, 2*2100]: per side, delta groups at WOFF offsets,
    # device channel order [k5|k4|k3|k2]
    wall = np.zeros((D, 2 * WSIDE), dtype=np.float32)
    bias_dev = np.zeros(640, dtype=np.float32)
    for k in (2, 3, 4, 5):
        blk = ORD_OF_K[k] * 150
        bias_dev[blk : blk + 150] = bs[k]
        for di, (delta, sz) in enumerate(DELTAS):
            t = delta + PAD_OF_K[k]
            if not (0 <= t < k):
                continue
            assert blk + 150 <= sz
            wall[:, WOFF[di] + blk : WOFF[di] + blk + 150] = ws[k][:, :D, t].T
            wall[:, WSIDE + WOFF[di] + blk : WSIDE + WOFF[di] + blk + 150] = (
                ws[k][:, D:, t].T
            )
    perm = np.zeros(2 * WSIDE, dtype=np.int64)
    for side in range(2):
        for di, (_, sz) in enumerate(DELTAS):
            for cc in range(5):
                c0 = CC0[cc]
                if sz <= c0:
                    continue
                w = min(128, sz - c0)
                newc = WCOL[(cc, side, di)]
                oldc = side * WSIDE + WOFF[di] + c0
                perm[newc : newc + w] = np.arange(oldc, oldc + w)
    wall = wall[:, perm].astype(BF16)
    bias_col = bias_dev[:, None]

    in_maps = []
    for core in range(8):
        b, jh = core // 2, core % 2
        jidx = [min(jh * JN + t, NQROW - 1) for t in range(JN)]
        psa = np.zeros((D, PS_COLS), dtype=np.float32)
        pqa = np.zeros((D, PQ_COLS), dtype=np.float32)
        for r in range(NROW):
            psa[:, r * ROWSTR + 4 : r * ROWSTR + 4 + L] = s[b, r].T
        for t, j in enumerate(jidx):
            pqa[:, t * ROWSTR + 4 : t * ROWSTR + 4 + L] = q[b, j].T
        in_maps.append(
            {
                "ps": psa.astype(BF16),
                "pq": pqa.astype(BF16),
                "w": wall,
                "bias": bias_col,
            }
        )
    return in_maps


# device channel -> original output channel maps
_S_IDX = np.array(
    [(3 - g) * 150 + u for g in range(4) for u in range(75)], dtype=np.int64
)
_Q_IDX = _S_IDX + 75


def assemble_outputs(core_outs):
    """core_outs: list of 8 arrays [NCH, NPAIR] -> (s_out, q_out)."""
    s_out = np.empty((B, NROW, NQROW, 300), dtype=np.float32)
    q_out = np.empty((B, NROW, NQROW, 300), dtype=np.float32)
    for core in range(8):
        b, jh = core // 2, core % 2
        nj = JN if jh == 0 else NQROW - JN
        # out[ch, j*25+i] -> [j, i, ch]
        arr = (
            np.asarray(core_outs[core])
            .astype(np.float32)
            .reshape(NCH, JN, NROW)
            .transpose(1, 2, 0)
        )
        s_out[b, :, jh * JN : jh * JN + nj] = arr[:nj][:, :, _S_IDX].transpose(
            1, 0, 2
        )
        q_out[b, :, jh * JN : jh * JN + nj] = arr[:nj][:, :, _Q_IDX].transpose(
            1, 0, 2
        )
    return s_out.reshape(-1, 300), q_out.reshape(-1, 300)


def kernel(s, q, w2, b2, w3, b3, w4, b4, w5, b5, B=4, N=5, K=5, Q=5, L=31):
    ws = {2: np.asarray(w2, np.float32), 3: np.asarray(w3, np.float32),
          4: np.asarray(w4, np.float32), 5: np.asarray(w5, np.float32)}
    bs = {2: np.asarray(b2, np.float32), 3: np.asarray(b3, np.float32),
          4: np.asarray(b4, np.float32), 5: np.asarray(b5, np.float32)}
    in_maps = build_inputs(s, q, ws, bs)
    nc = get_program()
    res = run_bass_kernel_spmd(nc, in_maps, list(range(8))).results
    return assemble_outputs([res[c]["out"] for c in range(8)])


# revision 8
# speedup vs baseline: 1.5156x; 1.0195x over previous
"""Trainium2 Bass kernel for nn_CNNInteractLayer (CNN interaction layer).

Math: for each episode b, s-row i, q-row j:
  out[b,i,j] = maxpool_L(relu(conv_k(concat(s[b,i], q[b,j])))) for k in 2..5
Factorization: conv(concat(s,q)) = conv_s(s) + conv_q(q) + bias, so per-row
convolutions run once on the PE (bf16, fp32 psum), and the pairwise stage is
fused on the vector engines: the scalar engine evicts conv psum to SBUF bf16
(s-side with bias folded in), the DVE forms pairwise sums with a broadcast
tensor_tensor add (2x bf16 mode) plus the first max-tree level, and the
otherwise-idle GPSIMD engine finishes the max tree. No pairwise matmul, no
A-matrix, no DRAM transpose roundtrip.

Sharding: 8 cores = 4 episodes x 2 halves of the q-row range.
"""

import os
import sys

import numpy as np

for _p in ("/opt/trn_rl_repo",):
    if os.path.isdir(_p) and _p not in sys.path:
        sys.path.insert(0, _p)

# the bass runner needs the axon jax backend; don't let a cpu-only pin hide it
if "axon" not in os.environ.get("JAX_PLATFORMS", "axon"):
    os.environ.pop("JAX_PLATFORMS", None)

import ml_dtypes  # noqa: E402

from concourse import bacc, bass, mybir, tile  # noqa: E402
from concourse.bass_utils import run_bass_kernel_spmd  # noqa: E402

BF16 = np.dtype(ml_dtypes.bfloat16)

# Problem dims (hardcoded per spec)
B, N, K, Q, L, D = 4, 5, 5, 5, 31, 512
NROW = N * K            # 25 s-rows per episode
NQROW = N * Q           # 25 q-rows per episode
JN = 13                 # q-rows per core (padded; odd cores use 12)
ROWSTR = L + 2          # 33: 2-zero gap between rows gives conv zero-padding
PS_COLS = NROW * ROWSTR + 9   # 834 (row r data at r*33+4 .. +34; halo right)
PQ_COLS = JN * ROWSTR + 9     # 438
S_OUT = 826             # conv output positions computed, s side (even)
Q_OUT = 430             # q side (even)
SLAB_S = 828            # per-chunk slab stride in cs_sb
SLAB_Q = 432
NCH = 600               # device channels: [k5 | k4 | k3 | k2] x 150
# delta (tap shift) groups; prefix-size in device channel order
DELTAS = [(-2, 300), (-1, 600), (0, 600), (1, 450), (2, 150)]
# emission order per d-chunk: full-coverage groups first so the first matmul
# of each PSUM accumulation group writes the full partition range
DORDER = [1, 2, 0, 3, 4]
WOFF = [0, 300, 900, 1500, 1950]  # packed col offset of each delta group
WSIDE = 2100
CC0 = [0, 128, 256, 384, 512]     # channel chunk starts
CCW = [128, 128, 128, 128, 88]
NPAIR = NROW * JN                 # 325
PAD_OF_K = {2: 1, 3: 1, 4: 2, 5: 2}
ORD_OF_K = {5: 0, 4: 1, 3: 2, 2: 3}
POSCH_S = [(0, 430), (430, 396)]  # row-aligned: rows 0-12 | 13-24
POSCH_Q = [(0, 430)]
# conv emission order: smallest chunks first to fill the DVE/Pool pipe early
CCORDER = [4, 3, 2, 1, 0]
JBLOCKS = [(0, 13)]  # pairwise j-blocks (single: fewer DVE instr inits)

# chunk-major packed-W layout: per channel chunk, [side s | side q], each a
# concatenation of the valid delta groups' column slices for that chunk
def _chunk_tables():
    chw = []          # per-side width of each chunk block
    coloff = {}       # (cc, side, di) -> column offset in packed W
    off = 0
    for cc in range(5):
        c0 = CC0[cc]
        widths = []
        for di, (_, sz) in enumerate(DELTAS):
            w = min(128, sz - c0) if sz > c0 else 0
            widths.append(w)
        side_w = sum(widths)
        for side in range(2):
            p = off + side * side_w
            for di, w in enumerate(widths):
                if w:
                    coloff[(cc, side, di)] = p
                    p += w
        chw.append(side_w)
        off += 2 * side_w
    return chw, coloff


CHW, WCOL = _chunk_tables()
CHOFF = [sum(2 * w for w in CHW[:i]) for i in range(6)]

_PROG = None


def _build_program():
    nc = bacc.Bacc("TRN2", target_bir_lowering=False, debug=False, num_devices=8)
    f32 = mybir.dt.float32
    bf16 = mybir.dt.bfloat16

    ps_d = nc.dram_tensor("ps", [D, PS_COLS], bf16, kind="ExternalInput")
    pq_d = nc.dram_tensor("pq", [D, PQ_COLS], bf16, kind="ExternalInput")
    w_d = nc.dram_tensor("w", [D, 2 * WSIDE], bf16, kind="ExternalInput")
    bias_d = nc.dram_tensor("bias", [640, 1], f32, kind="ExternalInput")
    out_d = nc.dram_tensor("out", [NCH, NPAIR], bf16, kind="ExternalOutput")

    with tile.TileContext(nc) as tc:
        with (
            tc.tile_pool(name="persist", bufs=1) as big,
            tc.tile_pool(name="tmppool", bufs=2) as tmppool,
            tc.tile_pool(name="t16pool", bufs=2) as t16pool,
            tc.tile_pool(name="t8pool", bufs=2) as t8pool,
            tc.tile_pool(name="t4pool", bufs=2) as t4pool,
            tc.tile_pool(name="t2pool", bufs=2) as t2pool,
            tc.tile_pool(name="convps", bufs=3, space="PSUM") as convps,
        ):
            w_sb = big.tile([128, 4 * 2 * WSIDE], bf16, tag="w")
            ps_sb = big.tile([128, 4 * PS_COLS], bf16, tag="ps")
            pq_sb = big.tile([128, 4 * PQ_COLS], bf16, tag="pq")
            cs_sb = big.tile([128, 5 * SLAB_S], bf16, tag="cs")
            cq_sb = big.tile([128, 5 * SLAB_Q], bf16, tag="cq")
            bias_sb = big.tile([128, 6], f32, tag="bias")
            red = big.tile([128, 5 * NPAIR], bf16, tag="red")

            def wload(cc):
                wd = w_d[:].rearrange("(d p) c -> p d c", p=128)
                ws = w_sb[:].rearrange("p (d c) -> p d c", c=2 * WSIDE)
                nc.sync.dma_start(
                    ws[:, :, CHOFF[cc] : CHOFF[cc + 1]],
                    wd[:, :, CHOFF[cc] : CHOFF[cc + 1]],
                )

            wload(CCORDER[0])
            pq3 = pq_sb[:].rearrange("p (d c) -> p d c", c=PQ_COLS)
            qd3 = pq_d[:].rearrange("(d p) c -> p d c", p=128)
            for d in range(4):
                nc.sync.dma_start(pq3[:, d : d + 1, :], qd3[:, d : d + 1, :])
            ps3 = ps_sb[:].rearrange("p (d c) -> p d c", c=PS_COLS)
            pd3 = ps_d[:].rearrange("(d p) c -> p d c", p=128)
            for d in range(4):
                nc.sync.dma_start(ps3[:, d : d + 1, :], pd3[:, d : d + 1, :])
            # bias[c] at dram row c -> bias_sb[p, cc] for c = cc*128+p
            nc.sync.dma_start(
                bias_sb[:, 0:5],
                bass.AP(bias_d[:].tensor, 0, [[1, 128], [128, 5]]),
            )
            nc.vector.memset(bias_sb[:, 5:6], 0.0)
            # touch the activation table during the DMA prologue so the
            # 1.3us LoadActFuncSet is off the critical path
            nc.scalar.activation(
                bias_sb[:, 5:6], bias_sb[:, 5:6],
                mybir.ActivationFunctionType.Relu,
            )
            wload(CCORDER[1])

            def conv(cc, side):
                """Conv for channel chunk cc of one side -> cs/cq slab."""
                c0, ccw = CC0[cc], CCW[cc]
                src, dst, poschunks, cols, slab, bcol = (
                    (ps_sb, cs_sb, POSCH_S, PS_COLS, SLAB_S, cc)
                    if side == 0
                    else (pq_sb, cq_sb, POSCH_Q, PQ_COLS, SLAB_Q, 5)
                )
                src3 = src[:].rearrange("p (d c) -> p d c", c=cols)
                for pos0, pw in poschunks:
                    psum = convps.tile([128, 432], f32, tag="conv")
                    mms = []
                    for d in range(4):
                        # first and last matmul of the accumulation group must
                        # cover the full partition range (start/stop semantics
                        # are per-element), so full-size delta groups bracket
                        order = DORDER if d < 3 else [1, 0, 3, 4, 2]
                        for di in order:
                            delta, sz = DELTAS[di]
                            if sz <= c0:
                                continue
                            wcc = min(ccw, sz - c0)
                            mms.append((d, di, delta, wcc))
                    for idx, (d, di, delta, wcc) in enumerate(mms):
                        lcol = d * 2 * WSIDE + WCOL[(cc, side, di)]
                        rcol = pos0 + delta + 2
                        nc.tensor.matmul(
                            psum[0:wcc, 0:pw],
                            lhsT=w_sb[:, lcol : lcol + wcc],
                            rhs=src3[:, d, rcol : rcol + pw],
                            start=(idx == 0),
                            stop=(idx == len(mms) - 1),
                        )
                    # psum -> SBUF bf16 on the scalar engine; bias folds into
                    # the s side so each pairwise sum gets it exactly once
                    nc.scalar.add(
                        dst[0:ccw, cc * slab + pos0 : cc * slab + pos0 + pw],
                        psum[0:ccw, 0:pw],
                        bias_sb[0:ccw, bcol : bcol + 1],
                    )

            def pairwise(cc, last=False):
                """Fused pairwise add + maxpool for chunk cc.

                tmp[ch,j,i,l] = cs[ch,i,l] + cq[ch,j,l]   (DVE, 2x bf16)
                L1 16-wide max on DVE, L2..L5 on GPSIMD, relu on DVE.
                """
                ccw = CCW[cc]
                mx = mybir.AluOpType.max
                for j0, jb in JBLOCKS:
                    npr = jb * NROW
                    tmp = tmppool.tile([128, JN * NROW * 31], bf16, tag="tmp")
                    t16 = t16pool.tile([128, JN * NROW * 16], bf16, tag="t16")
                    t8 = t8pool.tile([128, JN * NROW * 8], bf16, tag="t8")
                    t4 = t4pool.tile([128, JN * NROW * 4], bf16, tag="t4")
                    t2 = t2pool.tile([128, JN * NROW * 2], bf16, tag="t2")
                    def ap(t, off, dims):
                        tap = t[:]
                        return bass.AP(
                            tap.tensor,
                            tap.offset + off,
                            [[tap.ap[0][0], ccw]] + dims,
                        )

                    for i0, ni in ((0, 13), (13, 12)):
                        nc.vector.tensor_tensor(
                            ap(tmp, i0 * 31, [[775, jb], [31, ni], [1, 31]]),
                            ap(cs_sb, cc * SLAB_S + 2 + i0 * ROWSTR,
                               [[0, jb], [ROWSTR, ni], [1, 31]]),
                            ap(cq_sb, cc * SLAB_Q + 2 + j0 * ROWSTR,
                               [[ROWSTR, jb], [0, ni], [1, 31]]),
                            op=mybir.AluOpType.add,
                        )
                    nc.vector.tensor_tensor(
                        ap(t16, 0, [[16, npr], [1, 16]]),
                        ap(tmp, 0, [[31, npr], [1, 16]]),
                        ap(tmp, 15, [[31, npr], [1, 16]]),
                        op=mx,
                    )
                    nc.vector.tensor_tensor(
                        ap(t8, 0, [[8, npr], [1, 8]]),
                        ap(t16, 0, [[16, npr], [1, 8]]),
                        ap(t16, 8, [[16, npr], [1, 8]]),
                        op=mx,
                    )
                    nc.vector.tensor_tensor(
                        ap(t4, 0, [[4, npr], [1, 4]]),
                        ap(t8, 0, [[8, npr], [1, 4]]),
                        ap(t8, 4, [[8, npr], [1, 4]]),
                        op=mx,
                    )
                    nc.vector.tensor_tensor(
                        ap(t2, 0, [[2, npr], [1, 2]]),
                        ap(t4, 0, [[4, npr], [1, 2]]),
                        ap(t4, 2, [[4, npr], [1, 2]]),
                        op=mx,
                    )
                    nc.vector.tensor_tensor(
                        ap(red, cc * NPAIR + j0 * NROW, [[1, npr]]),
                        ap(t2, 0, [[2, npr]]),
                        ap(t2, 1, [[2, npr]]),
                        op=mx,
                    )
                # relu: scalar engine usually; DVE for the last chunk so
                # the tail is not gated on the Act pipeline
                if last:
                    nc.vector.tensor_scalar_max(
                        red[0:ccw, cc * NPAIR : (cc + 1) * NPAIR],
                        red[0:ccw, cc * NPAIR : (cc + 1) * NPAIR],
                        0.0,
                    )
                else:
                    nc.scalar.activation(
                        red[0:ccw, cc * NPAIR : (cc + 1) * NPAIR],
                        red[0:ccw, cc * NPAIR : (cc + 1) * NPAIR],
                        mybir.ActivationFunctionType.Relu,
                    )
                nc.sync.dma_start(
                    out_d[CC0[cc] : CC0[cc] + ccw, :],
                    red[0:ccw, cc * NPAIR : (cc + 1) * NPAIR],
                )

            # software pipeline: conv leads pairwise by one chunk
            conv(CCORDER[0], 1)
            conv(CCORDER[0], 0)
            for k in range(1, 5):
                if k + 1 <= 4:
                    wload(CCORDER[k + 1])
                conv(CCORDER[k], 1)
                conv(CCORDER[k], 0)
                pairwise(CCORDER[k - 1])
            pairwise(CCORDER[4], last=True)

    nc.compile()
    return nc


def get_program():
    global _PROG
    if _PROG is None:
        _PROG = _build_program()
    return _PROG


def build_inputs(s, q, ws, bs):
    """Host-side shard prep. ws/bs: dicts k -> w(150, 1024, k) / b(150,).

    Returns in_maps. Core c handles episode c//2, q-row half c%2.
    """
    s = np.asarray(s, dtype=np.float32).reshape(B, NROW, L, D)
    q = np.asarray(q, dtype=np.float32).reshape(B, NQROW, L, D)

    # packed weights [D, 2*2100]: per side, delta groups at WOFF offsets,
    # device channel order [k5|k4|k3|k2]
    wall = np.zeros((D, 2 * WSIDE), dtype=np.float32)
    bias_dev = np.zeros(640, dtype=np.float32)
    for k in (2, 3, 4, 5):
        blk = ORD_OF_K[k] * 150
        bias_dev[blk : blk + 150] = bs[k]
        for di, (delta, sz) in enumerate(DELTAS):
            t = delta + PAD_OF_K[k]
            if not (0 <= t < k):
                continue
            assert blk + 150 <= sz
            wall[:, WOFF[di] + blk : WOFF[di] + blk + 150] = ws[k][:, :D, t].T
            wall[:, WSIDE + WOFF[di] + blk : WSIDE + WOFF[di] + blk + 150] = (
                ws[k][:, D:, t].T
            )
    perm = np.zeros(2 * WSIDE, dtype=np.int64)
    for side in range(2):
        for di, (_, sz) in enumerate(DELTAS):
            for cc in range(5):
                c0 = CC0[cc]
                if sz <= c0:
                    continue
                w = min(128, sz - c0)
                newc = WCOL[(cc, side, di)]
                oldc = side * WSIDE + WOFF[di] + c0
                perm[newc : newc + w] = np.arange(oldc, oldc + w)
    wall = wall[:, perm].astype(BF16)
    bias_col = bias_dev[:, None]

    in_maps = []
    for core in range(8):
        b, jh = core // 2, core % 2
        jidx = [min(jh * JN + t, NQROW - 1) for t in range(JN)]
        psa = np.zeros((D, PS_COLS), dtype=np.float32)
        pqa = np.zeros((D, PQ_COLS), dtype=np.float32)
        for r in range(NROW):
            psa[:, r * ROWSTR + 4 : r * ROWSTR + 4 + L] = s[b, r].T
        for t, j in enumerate(jidx):
            pqa[:, t * ROWSTR + 4 : t * ROWSTR + 4 + L] = q[b, j].T
        in_maps.append(
            {
                "ps": psa.astype(BF16),
                "pq": pqa.astype(BF16),
                "w": wall,
                "bias": bias_col,
            }
        )
    return in_maps


# device channel -> original output channel maps
_S_IDX = np.array(
    [(3 - g) * 150 + u for g in range(4) for u in range(75)], dtype=np.int64
)
_Q_IDX = _S_IDX + 75


def assemble_outputs(core_outs):
    """core_outs: list of 8 arrays [NCH, NPAIR] -> (s_out, q_out)."""
    s_out = np.empty((B, NROW, NQROW, 300), dtype=np.float32)
    q_out = np.empty((B, NROW, NQROW, 300), dtype=np.float32)
    for core in range(8):
        b, jh = core // 2, core % 2
        nj = JN if jh == 0 else NQROW - JN
        # out[ch, j*25+i] -> [j, i, ch]
        arr = (
            np.asarray(core_outs[core])
            .astype(np.float32)
            .reshape(NCH, JN, NROW)
            .transpose(1, 2, 0)
        )
        s_out[b, :, jh * JN : jh * JN + nj] = arr[:nj][:, :, _S_IDX].transpose(
            1, 0, 2
        )
        q_out[b, :, jh * JN : jh * JN + nj] = arr[:nj][:, :, _Q_IDX].transpose(
            1, 0, 2
        )
    return s_out.reshape(-1, 300), q_out.reshape(-1, 300)


def kernel(s, q, w2, b2, w3, b3, w4, b4, w5, b5, B=4, N=5, K=5, Q=5, L=31):
    ws = {2: np.asarray(w2, np.float32), 3: np.asarray(w3, np.float32),
          4: np.asarray(w4, np.float32), 5: np.asarray(w5, np.float32)}
    bs = {2: np.asarray(b2, np.float32), 3: np.asarray(b3, np.float32),
          4: np.asarray(b4, np.float32), 5: np.asarray(b5, np.float32)}
    in_maps = build_inputs(s, q, ws, bs)
    nc = get_program()
    res = run_bass_kernel_spmd(nc, in_maps, list(range(8))).results
    return assemble_outputs([res[c]["out"] for c in range(8)])


# revision 9
# speedup vs baseline: 1.5323x; 1.0110x over previous
"""Trainium2 Bass kernel for nn_CNNInteractLayer (CNN interaction layer).

Math: for each episode b, s-row i, q-row j:
  out[b,i,j] = maxpool_L(relu(conv_k(concat(s[b,i], q[b,j])))) for k in 2..5
Factorization: conv(concat(s,q)) = conv_s(s) + conv_q(q) + bias, so per-row
convolutions run once on the PE (bf16, fp32 psum), and the pairwise stage is
fused on the vector engines: the scalar engine evicts conv psum to SBUF bf16
(s-side with bias folded in), the DVE forms pairwise sums with a broadcast
tensor_tensor add (2x bf16 mode) plus the first max-tree level, and the
otherwise-idle GPSIMD engine finishes the max tree. No pairwise matmul, no
A-matrix, no DRAM transpose roundtrip.

Sharding: 8 cores = 4 episodes x 2 halves of the q-row range.
"""

import os
import sys

import numpy as np

for _p in ("/opt/trn_rl_repo",):
    if os.path.isdir(_p) and _p not in sys.path:
        sys.path.insert(0, _p)

# the bass runner needs the axon jax backend; don't let a cpu-only pin hide it
if "axon" not in os.environ.get("JAX_PLATFORMS", "axon"):
    os.environ.pop("JAX_PLATFORMS", None)

import ml_dtypes  # noqa: E402

from concourse import bacc, bass, mybir, tile  # noqa: E402
from concourse.bass_utils import run_bass_kernel_spmd  # noqa: E402

BF16 = np.dtype(ml_dtypes.bfloat16)

# Problem dims (hardcoded per spec)
B, N, K, Q, L, D = 4, 5, 5, 5, 31, 512
NROW = N * K            # 25 s-rows per episode
NQROW = N * Q           # 25 q-rows per episode
JN = 13                 # q-rows per core (padded; odd cores use 12)
ROWSTR = L + 2          # 33: 2-zero gap between rows gives conv zero-padding
PS_COLS = NROW * ROWSTR + 9   # 834 (row r data at r*33+4 .. +34; halo right)
PQ_COLS = JN * ROWSTR + 9     # 438
S_OUT = 826             # conv output positions computed, s side (even)
Q_OUT = 430             # q side (even)
SLAB_S = 828            # per-chunk slab stride in cs_sb
SLAB_Q = 432
NCH = 600               # device channels: [k5 | k4 | k3 | k2] x 150
# delta (tap shift) groups; prefix-size in device channel order
DELTAS = [(-2, 300), (-1, 600), (0, 600), (1, 450), (2, 150)]
# emission order per d-chunk: full-coverage groups first so the first matmul
# of each PSUM accumulation group writes the full partition range
DORDER = [1, 2, 0, 3, 4]
WOFF = [0, 300, 900, 1500, 1950]  # packed col offset of each delta group
WSIDE = 2100
CC0 = [0, 128, 256, 384, 512]     # channel chunk starts
CCW = [128, 128, 128, 128, 88]
NPAIR = NROW * JN                 # 325
PAD_OF_K = {2: 1, 3: 1, 4: 2, 5: 2}
ORD_OF_K = {5: 0, 4: 1, 3: 2, 2: 3}
POSCH_S = [(0, 430), (430, 396)]  # row-aligned: rows 0-12 | 13-24
POSCH_Q = [(0, 430)]
# conv emission order: smallest chunks first to fill the DVE/Pool pipe early
CCORDER = [4, 3, 2, 1, 0]
JBLOCKS = [(0, 13)]  # pairwise j-blocks (single: fewer DVE instr inits)

# chunk-major packed-W layout: per channel chunk, [side s | side q], each a
# concatenation of the valid delta groups' column slices for that chunk
def _chunk_tables():
    chw = []          # per-side width of each chunk block
    coloff = {}       # (cc, side, di) -> column offset in packed W
    off = 0
    for cc in range(5):
        c0 = CC0[cc]
        widths = []
        for di, (_, sz) in enumerate(DELTAS):
            w = min(128, sz - c0) if sz > c0 else 0
            widths.append(w)
        side_w = sum(widths)
        for side in range(2):
            p = off + side * side_w
            for di, w in enumerate(widths):
                if w:
                    coloff[(cc, side, di)] = p
                    p += w
        chw.append(side_w)
        off += 2 * side_w
    return chw, coloff


CHW, WCOL = _chunk_tables()
CHOFF = [sum(2 * w for w in CHW[:i]) for i in range(6)]

_PROG = None


def _build_program():
    nc = bacc.Bacc("TRN2", target_bir_lowering=False, debug=False, num_devices=8)
    f32 = mybir.dt.float32
    bf16 = mybir.dt.bfloat16

    ps_d = nc.dram_tensor("ps", [D, PS_COLS], bf16, kind="ExternalInput")
    pq_d = nc.dram_tensor("pq", [D, PQ_COLS], bf16, kind="ExternalInput")
    w_d = nc.dram_tensor("w", [D, 2 * WSIDE], bf16, kind="ExternalInput")
    bias_d = nc.dram_tensor("bias", [640, 1], f32, kind="ExternalInput")
    out_d = nc.dram_tensor("out", [NCH, NPAIR], bf16, kind="ExternalOutput")

    with tile.TileContext(nc) as tc:
        with (
            tc.tile_pool(name="persist", bufs=1) as big,
            tc.tile_pool(name="tmppool", bufs=2) as tmppool,
            tc.tile_pool(name="t16pool", bufs=2) as t16pool,
            tc.tile_pool(name="t8pool", bufs=2) as t8pool,
            tc.tile_pool(name="t4pool", bufs=2) as t4pool,
            tc.tile_pool(name="t2pool", bufs=2) as t2pool,
            tc.tile_pool(name="convps", bufs=3, space="PSUM") as convps,
        ):
            w_sb = big.tile([128, 4 * 2 * WSIDE], bf16, tag="w")
            ps_sb = big.tile([128, 4 * PS_COLS], bf16, tag="ps")
            pq_sb = big.tile([128, 4 * PQ_COLS], bf16, tag="pq")
            cs_sb = big.tile([128, 5 * SLAB_S], bf16, tag="cs")
            cq_sb = big.tile([128, 5 * SLAB_Q], bf16, tag="cq")
            bias_sb = big.tile([128, 6], f32, tag="bias")
            red = big.tile([128, 5 * NPAIR], bf16, tag="red")

            def wload(cc):
                wd = w_d[:].rearrange("(d p) c -> p d c", p=128)
                ws = w_sb[:].rearrange("p (d c) -> p d c", c=2 * WSIDE)
                nc.sync.dma_start(
                    ws[:, :, CHOFF[cc] : CHOFF[cc + 1]],
                    wd[:, :, CHOFF[cc] : CHOFF[cc + 1]],
                )

            wload(CCORDER[0])
            pq3 = pq_sb[:].rearrange("p (d c) -> p d c", c=PQ_COLS)
            qd3 = pq_d[:].rearrange("(d p) c -> p d c", p=128)
            for d in range(4):
                nc.sync.dma_start(pq3[:, d : d + 1, :], qd3[:, d : d + 1, :])
            ps3 = ps_sb[:].rearrange("p (d c) -> p d c", c=PS_COLS)
            pd3 = ps_d[:].rearrange("(d p) c -> p d c", p=128)
            for d in range(4):
                # issue from the Act HWDGE queue: parallel with SP's w/pq DMAs
                nc.scalar.dma_start(ps3[:, d : d + 1, :], pd3[:, d : d + 1, :])
            # bias[c] at dram row c -> bias_sb[p, cc] for c = cc*128+p
            nc.scalar.dma_start(
                bias_sb[:, 0:5],
                bass.AP(bias_d[:].tensor, 0, [[1, 128], [128, 5]]),
            )
            nc.vector.memset(bias_sb[:, 5:6], 0.0)
            # touch the activation table during the DMA prologue so the
            # 1.3us LoadActFuncSet is off the critical path
            nc.scalar.activation(
                bias_sb[:, 5:6], bias_sb[:, 5:6],
                mybir.ActivationFunctionType.Relu,
            )
            wload(CCORDER[1])

            def conv(cc, side):
                """Conv for channel chunk cc of one side -> cs/cq slab."""
                c0, ccw = CC0[cc], CCW[cc]
                src, dst, poschunks, cols, slab, bcol = (
                    (ps_sb, cs_sb, POSCH_S, PS_COLS, SLAB_S, cc)
                    if side == 0
                    else (pq_sb, cq_sb, POSCH_Q, PQ_COLS, SLAB_Q, 5)
                )
                src3 = src[:].rearrange("p (d c) -> p d c", c=cols)
                for pos0, pw in poschunks:
                    psum = convps.tile([128, 432], f32, tag="conv")
                    mms = []
                    for d in range(4):
                        # first and last matmul of the accumulation group must
                        # cover the full partition range (start/stop semantics
                        # are per-element), so full-size delta groups bracket
                        order = DORDER if d < 3 else [1, 0, 3, 4, 2]
                        for di in order:
                            delta, sz = DELTAS[di]
                            if sz <= c0:
                                continue
                            wcc = min(ccw, sz - c0)
                            mms.append((d, di, delta, wcc))
                    for idx, (d, di, delta, wcc) in enumerate(mms):
                        lcol = d * 2 * WSIDE + WCOL[(cc, side, di)]
                        rcol = pos0 + delta + 2
                        nc.tensor.matmul(
                            psum[0:wcc, 0:pw],
                            lhsT=w_sb[:, lcol : lcol + wcc],
                            rhs=src3[:, d, rcol : rcol + pw],
                            start=(idx == 0),
                            stop=(idx == len(mms) - 1),
                        )
                    # psum -> SBUF bf16 on the scalar engine; bias folds into
                    # the s side so each pairwise sum gets it exactly once
                    nc.scalar.add(
                        dst[0:ccw, cc * slab + pos0 : cc * slab + pos0 + pw],
                        psum[0:ccw, 0:pw],
                        bias_sb[0:ccw, bcol : bcol + 1],
                    )

            def pairwise(cc, last=False):
                """Fused pairwise add + maxpool for chunk cc.

                tmp[ch,j,i,l] = cs[ch,i,l] + cq[ch,j,l]   (DVE, 2x bf16)
                L1 16-wide max on DVE, L2..L5 on GPSIMD, relu on DVE.
                """
                ccw = CCW[cc]
                mx = mybir.AluOpType.max
                for j0, jb in JBLOCKS:
                    npr = jb * NROW
                    tmp = tmppool.tile([128, JN * NROW * 31], bf16, tag="tmp")
                    t16 = t16pool.tile([128, JN * NROW * 16], bf16, tag="t16")
                    t8 = t8pool.tile([128, JN * NROW * 8], bf16, tag="t8")
                    t4 = t4pool.tile([128, JN * NROW * 4], bf16, tag="t4")
                    t2 = t2pool.tile([128, JN * NROW * 2], bf16, tag="t2")
                    def ap(t, off, dims):
                        tap = t[:]
                        return bass.AP(
                            tap.tensor,
                            tap.offset + off,
                            [[tap.ap[0][0], ccw]] + dims,
                        )

                    for i0, ni in ((0, 13), (13, 12)):
                        nc.vector.tensor_tensor(
                            ap(tmp, i0 * 31, [[775, jb], [31, ni], [1, 31]]),
                            ap(cs_sb, cc * SLAB_S + 2 + i0 * ROWSTR,
                               [[0, jb], [ROWSTR, ni], [1, 31]]),
                            ap(cq_sb, cc * SLAB_Q + 2 + j0 * ROWSTR,
                               [[ROWSTR, jb], [0, ni], [1, 31]]),
                            op=mybir.AluOpType.add,
                        )
                    nc.vector.tensor_tensor(
                        ap(t16, 0, [[16, npr], [1, 16]]),
                        ap(tmp, 0, [[31, npr], [1, 16]]),
                        ap(tmp, 15, [[31, npr], [1, 16]]),
                        op=mx,
                    )
                    nc.vector.tensor_tensor(
                        ap(t8, 0, [[8, npr], [1, 8]]),
                        ap(t16, 0, [[16, npr], [1, 8]]),
                        ap(t16, 8, [[16, npr], [1, 8]]),
                        op=mx,
                    )
                    nc.vector.tensor_tensor(
                        ap(t4, 0, [[4, npr], [1, 4]]),
                        ap(t8, 0, [[8, npr], [1, 4]]),
                        ap(t8, 4, [[8, npr], [1, 4]]),
                        op=mx,
                    )
                    nc.vector.tensor_tensor(
                        ap(t2, 0, [[2, npr], [1, 2]]),
                        ap(t4, 0, [[4, npr], [1, 2]]),
                        ap(t4, 2, [[4, npr], [1, 2]]),
                        op=mx,
                    )
                    nc.vector.tensor_tensor(
                        ap(red, cc * NPAIR + j0 * NROW, [[1, npr]]),
                        ap(t2, 0, [[2, npr]]),
                        ap(t2, 1, [[2, npr]]),
                        op=mx,
                    )
                # relu: scalar engine usually; DVE for the last chunk so
                # the tail is not gated on the Act pipeline, split in two so
                # the first out-DMA overlaps the second relu
                if last:
                    for p0, pn in ((0, 175), (175, 150)):
                        nc.vector.tensor_scalar_max(
                            red[0:ccw, cc * NPAIR + p0 : cc * NPAIR + p0 + pn],
                            red[0:ccw, cc * NPAIR + p0 : cc * NPAIR + p0 + pn],
                            0.0,
                        )
                        nc.sync.dma_start(
                            out_d[CC0[cc] : CC0[cc] + ccw, p0 : p0 + pn],
                            red[0:ccw, cc * NPAIR + p0 : cc * NPAIR + p0 + pn],
                        )
                else:
                    nc.scalar.activation(
                        red[0:ccw, cc * NPAIR : (cc + 1) * NPAIR],
                        red[0:ccw, cc * NPAIR : (cc + 1) * NPAIR],
                        mybir.ActivationFunctionType.Relu,
                    )
                    nc.sync.dma_start(
                        out_d[CC0[cc] : CC0[cc] + ccw, :],
                        red[0:ccw, cc * NPAIR : (cc + 1) * NPAIR],
                    )

            # software pipeline: conv leads pairwise by one chunk
            conv(CCORDER[0], 1)
            conv(CCORDER[0], 0)
            for k in range(1, 5):
                if k + 1 <= 4:
                    wload(CCORDER[k + 1])
                conv(CCORDER[k], 1)
                conv(CCORDER[k], 0)
                pairwise(CCORDER[k - 1])
            pairwise(CCORDER[4], last=True)

    nc.compile()
    return nc


def get_program():
    global _PROG
    if _PROG is None:
        _PROG = _build_program()
    return _PROG


def build_inputs(s, q, ws, bs):
    """Host-side shard prep. ws/bs: dicts k -> w(150, 1024, k) / b(150,).

    Returns in_maps. Core c handles episode c//2, q-row half c%2.
    """
    s = np.asarray(s, dtype=np.float32).reshape(B, NROW, L, D)
    q = np.asarray(q, dtype=np.float32).reshape(B, NQROW, L, D)

    # packed weights [D, 2*2100]: per side, delta groups at WOFF offsets,
    # device channel order [k5|k4|k3|k2]
    wall = np.zeros((D, 2 * WSIDE), dtype=np.float32)
    bias_dev = np.zeros(640, dtype=np.float32)
    for k in (2, 3, 4, 5):
        blk = ORD_OF_K[k] * 150
        bias_dev[blk : blk + 150] = bs[k]
        for di, (delta, sz) in enumerate(DELTAS):
            t = delta + PAD_OF_K[k]
            if not (0 <= t < k):
                continue
            assert blk + 150 <= sz
            wall[:, WOFF[di] + blk : WOFF[di] + blk + 150] = ws[k][:, :D, t].T
            wall[:, WSIDE + WOFF[di] + blk : WSIDE + WOFF[di] + blk + 150] = (
                ws[k][:, D:, t].T
            )
    perm = np.zeros(2 * WSIDE, dtype=np.int64)
    for side in range(2):
        for di, (_, sz) in enumerate(DELTAS):
            for cc in range(5):
                c0 = CC0[cc]
                if sz <= c0:
                    continue
                w = min(128, sz - c0)
                newc = WCOL[(cc, side, di)]
                oldc = side * WSIDE + WOFF[di] + c0
                perm[newc : newc + w] = np.arange(oldc, oldc + w)
    wall = wall[:, perm].astype(BF16)
    bias_col = bias_dev[:, None]

    in_maps = []
    for core in range(8):
        b, jh = core // 2, core % 2
        jidx = [min(jh * JN + t, NQROW - 1) for t in range(JN)]
        psa = np.zeros((D, PS_COLS), dtype=np.float32)
        pqa = np.zeros((D, PQ_COLS), dtype=np.float32)
        for r in range(NROW):
            psa[:, r * ROWSTR + 4 : r * ROWSTR + 4 + L] = s[b, r].T
        for t, j in enumerate(jidx):
            pqa[:, t * ROWSTR + 4 : t * ROWSTR + 4 + L] = q[b, j].T
        in_maps.append(
            {
                "ps": psa.astype(BF16),
                "pq": pqa.astype(BF16),
                "w": wall,
                "bias": bias_col,
            }
        )
    return in_maps


# device channel -> original output channel maps
_S_IDX = np.array(
    [(3 - g) * 150 + u for g in range(4) for u in range(75)], dtype=np.int64
)
_Q_IDX = _S_IDX + 75


def assemble_outputs(core_outs):
    """core_outs: list of 8 arrays [NCH, NPAIR] -> (s_out, q_out)."""
    s_out = np.empty((B, NROW, NQROW, 300), dtype=np.float32)
    q_out = np.empty((B, NROW, NQROW, 300), dtype=np.float32)
    for core in range(8):
        b, jh = core // 2, core % 2
        nj = JN if jh == 0 else NQROW - JN
        # out[ch, j*25+i] -> [j, i, ch]
        arr = (
            np.asarray(core_outs[core])
            .astype(np.float32)
            .reshape(NCH, JN, NROW)
            .transpose(1, 2, 0)
        )
        s_out[b, :, jh * JN : jh * JN + nj] = arr[:nj][:, :, _S_IDX].transpose(
            1, 0, 2
        )
        q_out[b, :, jh * JN : jh * JN + nj] = arr[:nj][:, :, _Q_IDX].transpose(
            1, 0, 2
        )
    return s_out.reshape(-1, 300), q_out.reshape(-1, 300)


def kernel(s, q, w2, b2, w3, b3, w4, b4, w5, b5, B=4, N=5, K=5, Q=5, L=31):
    ws = {2: np.asarray(w2, np.float32), 3: np.asarray(w3, np.float32),
          4: np.asarray(w4, np.float32), 5: np.asarray(w5, np.float32)}
    bs = {2: np.asarray(b2, np.float32), 3: np.asarray(b3, np.float32),
          4: np.asarray(b4, np.float32), 5: np.asarray(b5, np.float32)}
    in_maps = build_inputs(s, q, ws, bs)
    nc = get_program()
    res = run_bass_kernel_spmd(nc, in_maps, list(range(8))).results
    return assemble_outputs([res[c]["out"] for c in range(8)])
